# revision 2
# baseline (speedup 1.0000x reference)
"""Trainium2 Bass kernel for CrossModalAttention.

Reference computation (see problem):
  out = spatial + freq + CA(spatial->freq) + CA(freq->spatial)
with CA(q_src, kv_src) a multi-head (8 heads, d=32) cross-attention over
N = 64*64 = 4096 positions, C = 256 channels, plus 1x1-conv (channel matmul)
q/k/v/o projections with biases, shared weights between the two CA calls.

Sharding (8 cores): core = (cross, b, head_group) with 2 crosses x 2 batches
x 2 head-groups (4 heads = 128 channels each).  Each core computes its 4
heads' q/k/v projections, attention, and a partial output projection
(contracting only its 128 head-channels).  Host sums the two head-group
partials, adds residuals and the folded biases.

Bias algebra used (validated vs reference numerically):
  - bk drops entirely (softmax is invariant to per-query score offsets).
  - bv passes through softmax (weights sum to 1):  folded into host-side
    constant  wo @ bv.
  - bo added on host.
  - bq kept, applied on-device during the q projection.

On-device layout (per core):
  scoresT[n, m] = sum_d k[d, n] q[d, m]   (n on partitions -> PV matmul needs
  no transposes).  exp() is the bottleneck: it is split between the Scalar
  engine (exact exp activation, fused PSUM->SBUF drain) and the Vector engine
  (Schraudolph exp: bf16 bit pattern = round(s*A + B) computed as a single
  fused tensor_scalar into an int16 view).  Softmax denominators come from a
  ones-matmul accumulated with 32-row replication so the reciprocal is
  broadcast-free.  Normalization defers to after PV (linearity).
"""

import math
import os
import sys

import numpy as np

for _p in ("/opt/trn_rl_repo",):
    if _p not in sys.path and os.path.isdir(_p):
        sys.path.insert(0, _p)

import ml_dtypes

import concourse.bacc as bacc
import concourse.tile as tile
from concourse import mybir

P = 128          # partitions
HD = 32          # head dim
NH_CORE = 4      # heads per core
C = 256          # channels
KC = C // P      # contraction chunks for projections (2)
N_FULL = 4096    # H*W
SCALE = HD ** -0.5
MB = 512         # m-block (PSUM bank width in fp32)
NCH = 128        # n-chunk (partition dim of transposed scores)

# Schraudolph exp constants: bf16_bits(exp(SCALE*s)) ~= round(s*A + B)
A_SCH = SCALE * 128.0 / math.log(2.0)
B_SCH = 16256.0 - 5.61

F32 = mybir.dt.float32
BF16 = mybir.dt.bfloat16
I16 = mybir.dt.int16
EXP = mybir.ActivationFunctionType.Exp
MULT = mybir.AluOpType.mult
ADD = mybir.AluOpType.add


def emit(tc, nc, t, N, dve_mod=12, dve_k=5, st_tiles=3):
    """Emit the per-core program.  t: dict of DRAM APs."""
    from contextlib import ExitStack

    NB_M = N // MB
    NG = N // NCH
    STW = st_tiles * MB

    with ExitStack() as ctx:
        sb = ctx.enter_context(tc.tile_pool(name="sb", bufs=1))
        ps = ctx.enter_context(tc.tile_pool(name="ps", bufs=1, space="PSUM"))

        # ---- constants / inputs -> SBUF
        xq_sb = sb.tile([P, KC, N], BF16, name="xq_sb")
        xkv_sb = sb.tile([P, KC, N], BF16, name="xkv_sb")
        wq_sb = sb.tile([P, KC, P], BF16, name="wq_sb")
        wk_sb = sb.tile([P, KC, P], BF16, name="wk_sb")
        wv_sb = sb.tile([P, KC, P], BF16, name="wv_sb")
        wo_sb = sb.tile([P, C], BF16, name="wo_sb")
        bq_sb = sb.tile([P, 1], F32, name="bq_sb")
        ones_sb = sb.tile([P, HD], BF16, name="ones_sb")
        for kc in range(KC):
            nc.sync.dma_start(out=xq_sb[:, kc, :], in_=t["xq"][kc])
            nc.sync.dma_start(out=xkv_sb[:, kc, :], in_=t["xkv"][kc])
            nc.sync.dma_start(out=wq_sb[:, kc, :], in_=t["wqT"][kc])
            nc.sync.dma_start(out=wk_sb[:, kc, :], in_=t["wkT"][kc])
            nc.sync.dma_start(out=wv_sb[:, kc, :], in_=t["wvT"][kc])
        nc.sync.dma_start(out=wo_sb, in_=t["woT"])
        nc.sync.dma_start(out=bq_sb, in_=t["bq"])
        nc.vector.memset(ones_sb, 1.0)

        q_sb = sb.tile([P, N], BF16, name="q_sb")
        k_sb = sb.tile([P, N], BF16, name="k_sb")
        vT_sb = sb.tile([P, N], BF16, name="vT_sb")

        # ---- projections: q, k as (hd=128, pos); v transposed (pos, hd=128)
        for dst, w_sb, x_sb, bias in (
            (q_sb, wq_sb, xq_sb, bq_sb),
            (k_sb, wk_sb, xkv_sb, None),
        ):
            for lo in range(0, N, STW):
                hi = min(N, lo + STW)
                w = hi - lo
                pt = ps.tile([P, STW], F32, tag="qk", bufs=2,
                             name=f"prj_{dst.tensor.name}_{lo}")
                for kc in range(KC):
                    for j in range(lo, hi, MB):
                        nc.tensor.matmul(
                            pt[:, j - lo:j - lo + MB],
                            lhsT=w_sb[:, kc, :],
                            rhs=x_sb[:, kc, j:j + MB],
                            start=(kc == 0), stop=(kc == KC - 1),
                        )
                if bias is not None:
                    nc.vector.tensor_scalar(
                        out=dst[:, lo:hi], in0=pt[:, :w],
                        scalar1=bias, scalar2=None, op0=ADD,
                    )
                else:
                    nc.vector.tensor_copy(out=dst[:, lo:hi], in_=pt[:, :w])

        for lo in range(0, N, STW):
            hi = min(N, lo + STW)
            pt = ps.tile([P, STW], F32, tag="qk", bufs=2, name=f"prj_vt_{lo}")
            for g0 in range(lo, hi, P):
                for kc in range(KC):
                    nc.tensor.matmul(
                        pt[:, g0 - lo:g0 - lo + P],
                        lhsT=xkv_sb[:, kc, g0:g0 + P],
                        rhs=wv_sb[:, kc, :],
                        start=(kc == 0), stop=(kc == KC - 1),
                    )
            nc.vector.tensor_copy(out=vT_sb[:, lo:hi], in_=pt[:, :hi - lo])

        # ---- attention, per m-block of 512 query positions
        for mb in range(NB_M):
            m0 = mb * MB
            pv_ps = ps.tile([P, MB], F32, tag="pv", bufs=1, name=f"pv{mb}")
            dn_ps = ps.tile([P, MB], F32, tag="dn", bufs=1, name=f"dn{mb}")

            st_ps = st_sb = None
            fill = 0
            st_idx = 0
            pending = []
            for g in range(NG):
                for h in range(NH_CORE):
                    if fill == 0:
                        st_ps = ps.tile([P, STW], F32, tag="qk", bufs=2,
                                        name=f"stp{mb}_{st_idx}")
                        st_sb = sb.tile([P, STW], BF16, tag="attn", bufs=6,
                                        name=f"sts{mb}_{st_idx}")
                    sl = slice(fill * MB, (fill + 1) * MB)
                    # scoresT chunk: out[n, m] = sum_d k[d, n] * q[d, m]
                    nc.tensor.matmul(
                        st_ps[:, sl],
                        lhsT=k_sb[h * HD:(h + 1) * HD, g * NCH:(g + 1) * NCH],
                        rhs=q_sb[h * HD:(h + 1) * HD, m0:m0 + MB],
                        start=True, stop=True,
                        tile_position=(h * HD, 0),
                    )
                    pending.append((h, g, st_sb[:, sl]))
                    fill += 1
                    if fill == st_tiles or (g == NG - 1 and h == NH_CORE - 1):
                        w = fill * MB
                        if dve_k and (st_idx % dve_mod) < dve_k:
                            # Schraudolph exp on the Vector engine
                            nc.vector.tensor_scalar(
                                out=st_sb[:, :w].bitcast(I16),
                                in0=st_ps[:, :w],
                                scalar1=A_SCH, scalar2=B_SCH,
                                op0=MULT, op1=ADD,
                            )
                        else:
                            nc.scalar.activation(
                                out=st_sb[:, :w], in_=st_ps[:, :w],
                                func=EXP, scale=SCALE,
                            )
                        for hh, gg, ap in pending:
                            nc.tensor.matmul(
                                pv_ps[hh * HD:(hh + 1) * HD, :],
                                lhsT=vT_sb[:, gg * NCH + hh * HD:
                                           gg * NCH + (hh + 1) * HD],
                                rhs=ap,
                                start=(gg == 0), stop=(gg == NG - 1),
                                tile_position=(0, hh * HD),
                                skip_group_check=True,
                            )
                            nc.tensor.matmul(
                                dn_ps[hh * HD:(hh + 1) * HD, :],
                                lhsT=ones_sb,
                                rhs=ap,
                                start=(gg == 0), stop=(gg == NG - 1),
                                tile_position=(0, hh * HD),
                                skip_group_check=True,
                            )
                        pending = []
                        fill = 0
                        st_idx += 1

            recip_sb = sb.tile([P, MB], F32, tag="recip", bufs=2, name=f"rc{mb}")
            nc.vector.reciprocal_approx_fast(out=recip_sb, in_=dn_ps)
            y_sb = sb.tile([P, MB], BF16, tag="y", bufs=2, name=f"y{mb}")
            nc.vector.tensor_mul(y_sb, pv_ps, recip_sb)
            for j in range(2):
                op_ps = ps.tile([P, MB], F32, tag="pv", bufs=1,
                                name=f"op{mb}_{j}")
                nc.tensor.matmul(op_ps, lhsT=wo_sb[:, j * P:(j + 1) * P],
                                 rhs=y_sb, start=True, stop=True)
                o_sb = sb.tile([P, MB], F32, tag="osb", bufs=3,
                               name=f"ob{mb}_{j}")
                nc.vector.tensor_copy(out=o_sb, in_=op_ps)
                nc.sync.dma_start(out=t["o"][j, :, m0:m0 + MB], in_=o_sb)


def build_program(N=N_FULL, dve_mod=12, dve_k=5, st_tiles=3):
    nc = bacc.Bacc(
        "TRN2",
        target_bir_lowering=False,
        debug=False,
        enable_asserts=False,
    )
    t = {
        "xq": nc.dram_tensor("xq", [KC, P, N], BF16, kind="ExternalInput").ap(),
        "xkv": nc.dram_tensor("xkv", [KC, P, N], BF16, kind="ExternalInput").ap(),
        "wqT": nc.dram_tensor("wqT", [KC, P, P], BF16, kind="ExternalInput").ap(),
        "wkT": nc.dram_tensor("wkT", [KC, P, P], BF16, kind="ExternalInput").ap(),
        "wvT": nc.dram_tensor("wvT", [KC, P, P], BF16, kind="ExternalInput").ap(),
        "woT": nc.dram_tensor("woT", [P, C], BF16, kind="ExternalInput").ap(),
        "bq": nc.dram_tensor("bq", [P, 1], F32, kind="ExternalInput").ap(),
        "o": nc.dram_tensor("o", [2, P, N], F32, kind="ExternalOutput").ap(),
    }
    with tile.TileContext(nc) as tc:
        emit(tc, nc, t, N, dve_mod=dve_mod, dve_k=dve_k, st_tiles=st_tiles)
    nc.compile()
    return nc


def make_in_maps(spatial_feat, freq_feat, wq, bq, wk, bk, wv, bv, wo, bo, N=N_FULL):
    """Host-side sharding: returns the 8 per-core input dicts."""
    bf = ml_dtypes.bfloat16
    f32 = np.float32
    spatial = np.asarray(spatial_feat, f32).reshape(2, C, N)
    freq = np.asarray(freq_feat, f32).reshape(2, C, N)
    wq, wk, wv, wo = (np.asarray(a, f32) for a in (wq, wk, wv, wo))
    bq = np.asarray(bq, f32)
    in_maps = []
    for c in range(8):
        cross, b, hg = c >> 2, (c >> 1) & 1, c & 1
        qs = spatial if cross == 0 else freq
        kv = freq if cross == 0 else spatial
        hsl = slice(hg * P, (hg + 1) * P)
        in_maps.append({
            "xq": np.ascontiguousarray(qs[b]).astype(bf).reshape(KC, P, N),
            "xkv": np.ascontiguousarray(kv[b]).astype(bf).reshape(KC, P, N),
            "wqT": np.ascontiguousarray(wq[hsl, :].T).astype(bf).reshape(KC, P, P),
            "wkT": np.ascontiguousarray(wk[hsl, :].T).astype(bf).reshape(KC, P, P),
            "wvT": np.ascontiguousarray(wv[hsl, :].T).astype(bf).reshape(KC, P, P),
            "woT": np.ascontiguousarray(wo[:, hsl].T).astype(bf),
            "bq": np.ascontiguousarray(bq[hsl]).reshape(P, 1).astype(f32),
        })
    return in_maps


def combine(results, spatial_feat, freq_feat, wv, bv, wo, bo):
    """Host-side gather: sum head-group partials, add residuals + folded biases."""
    f32 = np.float32
    spatial = np.asarray(spatial_feat, f32).reshape(2, C, N_FULL)
    freq = np.asarray(freq_feat, f32).reshape(2, C, N_FULL)
    wv, bv, wo, bo = (np.asarray(a, f32) for a in (wv, bv, wo, bo))
    ca = np.zeros((2, 2, C, N_FULL), f32)  # [cross, b]
    for c in range(8):
        cross, b = c >> 2, (c >> 1) & 1
        ca[cross, b] += results[c]["o"].reshape(C, N_FULL)
    cbias = (bo + wo @ bv)[None, :, None]
    out = spatial + freq + ca[0] + ca[1] + 2.0 * cbias
    return out.reshape(2, C, 64, 64).astype(f32)


_NC_CACHE = {}


def _get_nc(**kw):
    key = tuple(sorted(kw.items()))
    if key not in _NC_CACHE:
        _NC_CACHE[key] = build_program(**kw)
    return _NC_CACHE[key]


def kernel(spatial_feat, freq_feat, wq, bq, wk, bk, wv, bv, wo, bo):
    from concourse.bass_utils import run_bass_kernel_spmd

    nc = _get_nc()
    in_maps = make_in_maps(spatial_feat, freq_feat, wq, bq, wk, bk, wv, bv, wo, bo)
    res = run_bass_kernel_spmd(nc, in_maps, list(range(8)))
    return combine(res.results, spatial_feat, freq_feat, wv, bv, wo, bo)


# revision 16
# speedup vs baseline: 1.9575x; 1.9575x over previous
"""Trainium2 Bass kernel for CrossModalAttention.

Reference computation (see problem):
  out = spatial + freq + CA(spatial->freq) + CA(freq->spatial)
with CA(q_src, kv_src) a multi-head (8 heads, d=32) cross-attention over
N = 64*64 = 4096 positions, C = 256 channels, plus 1x1-conv (channel matmul)
q/k/v/o projections with biases, shared weights between the two CA calls.

Sharding (8 cores): core = (cross, b, head_group) with 2 crosses x 2 batches
x 2 head-groups (4 heads = 128 channels each).  Each core computes its 4
heads' q/k/v projections, attention, and a partial output projection
(contracting only its 128 head-channels).  Host sums the two head-group
partials, adds residuals and the folded biases.

Bias algebra used (validated vs reference numerically):
  - bk drops entirely (softmax is invariant to per-query score offsets).
  - bv passes through softmax (weights sum to 1):  folded into host-side
    constant  wo @ bv.
  - bo added on host.
  - bq kept, applied on-device during the q projection.

On-device layout (per core):
  scoresT[n, m] = sum_d k[d, n] q[d, m]   (n on partitions -> PV matmul needs
  no transposes).  exp() is the bottleneck: it is split between the Scalar
  engine (exact exp activation, fused PSUM->SBUF drain) and the Vector engine
  (Schraudolph exp: bf16 bit pattern = round(s*A + B) computed as a single
  fused tensor_scalar into an int16 view).  Softmax denominators come from a
  ones-matmul accumulated with 32-row replication so the reciprocal is
  broadcast-free.  Normalization defers to after PV (linearity).
"""

import math
import os
import sys

import numpy as np

for _p in ("/opt/trn_rl_repo",):
    if _p not in sys.path and os.path.isdir(_p):
        sys.path.insert(0, _p)

import ml_dtypes

import concourse.bacc as bacc
import concourse.tile as tile
from concourse import mybir

P = 128          # partitions
HD = 32          # head dim
NH_CORE = 4      # heads per core
C = 256          # channels
KC = C // P      # contraction chunks for projections (2)
N_FULL = 4096    # H*W
SCALE = HD ** -0.5
MB = 512         # m-block (PSUM bank width in fp32)
NCH = 128        # n-chunk (partition dim of transposed scores)

# Schraudolph exp constants: bf16_bits(exp(SCALE*s)) ~= round(s*A + B)
A_SCH = SCALE * 128.0 / math.log(2.0)
B_SCH = 16256.0 - 5.61

F32 = mybir.dt.float32
BF16 = mybir.dt.bfloat16
I16 = mybir.dt.int16
EXP = mybir.ActivationFunctionType.Exp
MULT = mybir.AluOpType.mult
ADD = mybir.AluOpType.add


def emit(tc, nc, t, N, dve_mod=12, dve_k=5, st_tiles=2):
    """Emit the per-core program.  t: dict of DRAM APs."""
    from contextlib import ExitStack

    NB_M = N // MB
    NG = N // NCH
    STW = st_tiles * MB

    with ExitStack() as ctx:
        sb = ctx.enter_context(tc.tile_pool(name="sb", bufs=1))
        ps = ctx.enter_context(tc.tile_pool(name="ps", bufs=1, space="PSUM"))

        # ---- constants / inputs -> SBUF
        xq_sb = sb.tile([P, KC, N], BF16, name="xq_sb")
        xkv_sb = sb.tile([P, KC, N], BF16, name="xkv_sb")
        wq_sb = sb.tile([P, KC, P], BF16, name="wq_sb")
        wk_sb = sb.tile([P, KC, P], BF16, name="wk_sb")
        wv_sb = sb.tile([P, KC, P], BF16, name="wv_sb")
        wo_sb = sb.tile([P, C], BF16, name="wo_sb")
        bq_sb = sb.tile([P, 1], F32, name="bq_sb")
        ones_sb = sb.tile([P, HD], BF16, name="ones_sb")
        for kc in range(KC):
            nc.sync.dma_start(out=xq_sb[:, kc, :], in_=t["xq"][kc])
            nc.sync.dma_start(out=xkv_sb[:, kc, :], in_=t["xkv"][kc])
            nc.sync.dma_start(out=wq_sb[:, kc, :], in_=t["wqT"][kc])
            nc.sync.dma_start(out=wk_sb[:, kc, :], in_=t["wkT"][kc])
            nc.sync.dma_start(out=wv_sb[:, kc, :], in_=t["wvT"][kc])
        nc.sync.dma_start(out=wo_sb, in_=t["woT"])
        nc.sync.dma_start(out=bq_sb, in_=t["bq"])
        nc.vector.memset(ones_sb, 1.0)

        q_sb = sb.tile([P, N], BF16, name="q_sb")
        k_sb = sb.tile([P, N], BF16, name="k_sb")
        vT_sb = sb.tile([P, N], BF16, name="vT_sb")

        # ---- projections: q, k as (hd=128, pos); v transposed (pos, hd=128)
        for dst, w_sb, x_sb, bias in (
            (q_sb, wq_sb, xq_sb, bq_sb),
            (k_sb, wk_sb, xkv_sb, None),
        ):
            for lo in range(0, N, STW):
                hi = min(N, lo + STW)
                w = hi - lo
                pt = ps.tile([P, STW], F32, tag="qk", bufs=3,
                             name=f"prj_{dst.tensor.name}_{lo}")
                for kc in range(KC):
                    for j in range(lo, hi, MB):
                        nc.tensor.matmul(
                            pt[:, j - lo:j - lo + MB],
                            lhsT=w_sb[:, kc, :],
                            rhs=x_sb[:, kc, j:j + MB],
                            start=(kc == 0), stop=(kc == KC - 1),
                        )
                if bias is not None:
                    nc.vector.tensor_scalar(
                        out=dst[:, lo:hi], in0=pt[:, :w],
                        scalar1=bias, scalar2=None, op0=ADD,
                    )
                else:
                    nc.vector.tensor_copy(out=dst[:, lo:hi], in_=pt[:, :w])

        for lo in range(0, N, STW):
            hi = min(N, lo + STW)
            pt = ps.tile([P, STW], F32, tag="qk", bufs=3, name=f"prj_vt_{lo}")
            for g0 in range(lo, hi, P):
                for kc in range(KC):
                    nc.tensor.matmul(
                        pt[:, g0 - lo:g0 - lo + P],
                        lhsT=xkv_sb[:, kc, g0:g0 + P],
                        rhs=wv_sb[:, kc, :],
                        start=(kc == 0), stop=(kc == KC - 1),
                    )
            nc.vector.tensor_copy(out=vT_sb[:, lo:hi], in_=pt[:, :hi - lo])

        # ---- attention, per m-block of 512 query positions
        # The previous m-block's output projection is deferred into the next
        # m-block's stream (after 2 supertiles) so this m-block's QK packs sit
        # ahead of it in the PE FIFO; its PSUM comes from the "dn" tag, whose
        # slot is free right after the reciprocal reads it.
        deferred_tail = None
        n_st_total = (NG * NH_CORE + st_tiles - 1) // st_tiles
        for mb in range(NB_M):
            m0 = mb * MB
            pv_ps = ps.tile([P, MB], F32, tag="pv", bufs=1, name=f"pv{mb}")
            dn_ps = None
            state = {"dn": None}

            st_ps = st_sb = None
            fill = 0
            st_idx = 0
            pending = []   # tiles of the current supertile (pre-exp)
            pv_queue = []  # exp'd tiles awaiting PV/denominator emission

            def drain_pv(nd=None):
                # two 4-concurrent col-group waves: all pv, then all dn.
                # Called with nd=4 while streaming (drains the oldest n-chunk,
                # lagging one supertile so the wave never waits at the PE
                # queue head); nd=None flushes.
                dn = state["dn"]
                batch = pv_queue[:nd] if nd else list(pv_queue)
                del pv_queue[:len(batch)]
                for hh, gg, ap in batch:
                    nc.tensor.matmul(
                        pv_ps[hh * HD:(hh + 1) * HD, :],
                        lhsT=vT_sb[:, gg * NCH + hh * HD:
                                   gg * NCH + (hh + 1) * HD],
                        rhs=ap,
                        start=(gg == 0), stop=(gg == NG - 1),
                        tile_position=(0, hh * HD),
                        skip_group_check=True,
                    )
                for hh, gg, ap in batch:
                    nc.tensor.matmul(
                        dn[hh * HD:(hh + 1) * HD, :],
                        lhsT=ones_sb,
                        rhs=ap,
                        start=(gg == 0), stop=(gg == NG - 1),
                        tile_position=(0, hh * HD),
                        skip_group_check=True,
                    )

            for g in range(NG):
                for h in range(NH_CORE):
                    if fill == 0 and (st_idx == 2 or n_st_total <= 2):
                        if deferred_tail is not None:
                            deferred_tail()
                            deferred_tail = None
                        state["dn"] = ps.tile([P, MB], F32, tag="dn", bufs=1,
                                              name=f"dn{mb}")
                    if fill == 0:
                        st_ps = ps.tile([P, STW], F32, tag="qk", bufs=3,
                                        name=f"stp{mb}_{st_idx}")
                        st_sb = sb.tile([P, STW], BF16, tag="attn", bufs=24,
                                        name=f"sts{mb}_{st_idx}")
                    sl = slice(fill * MB, (fill + 1) * MB)
                    # scoresT chunk: out[n, m] = sum_d k[d, n] * q[d, m]
                    nc.tensor.matmul(
                        st_ps[:, sl],
                        lhsT=k_sb[h * HD:(h + 1) * HD, g * NCH:(g + 1) * NCH],
                        rhs=q_sb[h * HD:(h + 1) * HD, m0:m0 + MB],
                        start=True, stop=True,
                        tile_position=(h * HD, 0),
                    )
                    pending.append((h, g, st_sb[:, sl]))
                    fill += 1
                    if fill == st_tiles or (g == NG - 1 and h == NH_CORE - 1):
                        w = fill * MB
                        # Bresenham-interleaved engine split so ACT and DVE
                        # drain supertiles concurrently
                        if dve_k and (st_idx * dve_k) % dve_mod < dve_k:
                            # Schraudolph exp on the Vector engine
                            nc.vector.tensor_scalar(
                                out=st_sb[:, :w].bitcast(I16),
                                in0=st_ps[:, :w],
                                scalar1=A_SCH, scalar2=B_SCH,
                                op0=MULT, op1=ADD,
                            )
                        else:
                            nc.scalar.activation(
                                out=st_sb[:, :w], in_=st_ps[:, :w],
                                func=EXP, scale=SCALE,
                            )
                        pv_queue.extend(pending)
                        pending = []
                        if state["dn"] is not None and len(pv_queue) >= 6 * NH_CORE:
                            drain_pv(NH_CORE)
                        fill = 0
                        st_idx += 1

            drain_pv()
            dn_ps = state["dn"]
            recip_sb = sb.tile([P, MB], F32, tag="recip", bufs=2, name=f"rc{mb}")
            nc.vector.reciprocal_approx_fast(out=recip_sb, in_=dn_ps)
            y_sb = sb.tile([P, MB], BF16, tag="y", bufs=2, name=f"y{mb}")
            nc.vector.tensor_mul(y_sb, pv_ps, recip_sb)

            def make_tail(mb=mb, m0=m0, y_sb=y_sb):
                def tail():
                    for j in range(2):
                        op_ps = ps.tile([P, MB], F32, tag="dn", bufs=1,
                                        name=f"op{mb}_{j}")
                        nc.tensor.matmul(op_ps,
                                         lhsT=wo_sb[:, j * P:(j + 1) * P],
                                         rhs=y_sb, start=True, stop=True)
                        o_sb = sb.tile([P, MB], F32, tag="osb", bufs=3,
                                       name=f"ob{mb}_{j}")
                        nc.vector.tensor_copy(out=o_sb, in_=op_ps)
                        nc.sync.dma_start(out=t["o"][j, :, m0:m0 + MB],
                                          in_=o_sb)
                return tail

            deferred_tail = make_tail()
        if deferred_tail is not None:
            deferred_tail()


def build_program(N=N_FULL, dve_mod=12, dve_k=5, st_tiles=2):
    nc = bacc.Bacc(
        "TRN2",
        target_bir_lowering=False,
        debug=False,
        enable_asserts=False,
    )
    t = {
        "xq": nc.dram_tensor("xq", [KC, P, N], BF16, kind="ExternalInput").ap(),
        "xkv": nc.dram_tensor("xkv", [KC, P, N], BF16, kind="ExternalInput").ap(),
        "wqT": nc.dram_tensor("wqT", [KC, P, P], BF16, kind="ExternalInput").ap(),
        "wkT": nc.dram_tensor("wkT", [KC, P, P], BF16, kind="ExternalInput").ap(),
        "wvT": nc.dram_tensor("wvT", [KC, P, P], BF16, kind="ExternalInput").ap(),
        "woT": nc.dram_tensor("woT", [P, C], BF16, kind="ExternalInput").ap(),
        "bq": nc.dram_tensor("bq", [P, 1], F32, kind="ExternalInput").ap(),
        "o": nc.dram_tensor("o", [2, P, N], F32, kind="ExternalOutput").ap(),
    }
    with tile.TileContext(nc) as tc:
        emit(tc, nc, t, N, dve_mod=dve_mod, dve_k=dve_k, st_tiles=st_tiles)
    nc.compile()
    return nc


def make_in_maps(spatial_feat, freq_feat, wq, bq, wk, bk, wv, bv, wo, bo, N=N_FULL):
    """Host-side sharding: returns the 8 per-core input dicts."""
    bf = ml_dtypes.bfloat16
    f32 = np.float32
    spatial = np.asarray(spatial_feat, f32).reshape(2, C, N)
    freq = np.asarray(freq_feat, f32).reshape(2, C, N)
    wq, wk, wv, wo = (np.asarray(a, f32) for a in (wq, wk, wv, wo))
    bq = np.asarray(bq, f32)
    in_maps = []
    for c in range(8):
        cross, b, hg = c >> 2, (c >> 1) & 1, c & 1
        qs = spatial if cross == 0 else freq
        kv = freq if cross == 0 else spatial
        hsl = slice(hg * P, (hg + 1) * P)
        in_maps.append({
            "xq": np.ascontiguousarray(qs[b]).astype(bf).reshape(KC, P, N),
            "xkv": np.ascontiguousarray(kv[b]).astype(bf).reshape(KC, P, N),
            "wqT": np.ascontiguousarray(wq[hsl, :].T).astype(bf).reshape(KC, P, P),
            "wkT": np.ascontiguousarray(wk[hsl, :].T).astype(bf).reshape(KC, P, P),
            "wvT": np.ascontiguousarray(wv[hsl, :].T).astype(bf).reshape(KC, P, P),
            "woT": np.ascontiguousarray(wo[:, hsl].T).astype(bf),
            "bq": np.ascontiguousarray(bq[hsl]).reshape(P, 1).astype(f32),
        })
    return in_maps


def combine(results, spatial_feat, freq_feat, wv, bv, wo, bo):
    """Host-side gather: sum head-group partials, add residuals + folded biases."""
    f32 = np.float32
    spatial = np.asarray(spatial_feat, f32).reshape(2, C, N_FULL)
    freq = np.asarray(freq_feat, f32).reshape(2, C, N_FULL)
    wv, bv, wo, bo = (np.asarray(a, f32) for a in (wv, bv, wo, bo))
    ca = np.zeros((2, 2, C, N_FULL), f32)  # [cross, b]
    for c in range(8):
        cross, b = c >> 2, (c >> 1) & 1
        ca[cross, b] += results[c]["o"].reshape(C, N_FULL)
    cbias = (bo + wo @ bv)[None, :, None]
    out = spatial + freq + ca[0] + ca[1] + 2.0 * cbias
    return out.reshape(2, C, 64, 64).astype(f32)


_NC_CACHE = {}


def _get_nc(**kw):
    key = tuple(sorted(kw.items()))
    if key not in _NC_CACHE:
        _NC_CACHE[key] = build_program(**kw)
    return _NC_CACHE[key]


def kernel(spatial_feat, freq_feat, wq, bq, wk, bk, wv, bv, wo, bo):
    from concourse.bass_utils import run_bass_kernel_spmd

    nc = _get_nc()
    in_maps = make_in_maps(spatial_feat, freq_feat, wq, bq, wk, bk, wv, bv, wo, bo)
    res = run_bass_kernel_spmd(nc, in_maps, list(range(8)))
    return combine(res.results, spatial_feat, freq_feat, wv, bv, wo, bo)
